# revision 1
# baseline (speedup 1.0000x reference)
"""Trainium2 Bass kernel for a 2D correlation layer.

out[b, dx*41+dy, h, w] = sum_c x[b,c,h,w] * xpad[b,c,h+dx,w+dy]
with x of shape (4, 256, 64, 128), 41x41 displacements (max_disp 20).

Strategy (8 NeuronCores, SPMD):
  - Shard by (batch b, h-half): core = b*2 + half, each core computes
    out[b, :, H0:H0+32, :] for H0 = 32*half.
  - Host pre-pads each core's input to a bf16 slab [256, 72, 168];
    bf16 matmuls stream 1 PE cycle/column (4x over fp32).
  - Weight-rect matmuls with M=128: each rect = 8 h-rows x 16 w-pixels
    of x as stationary weights (gathered contiguous per rect), rhs =
    the slab window [s in 48 rows, u in 56 cols], contracting C=256 in
    two K=128 passes into PSUM S[(rh, wl), (s, u)].  All 8 w-tiles of
    an h-octet accumulate into ONE big SBUF tile [128, 8*2688] (bf16).
  - The displacement shear (dx = s - rh, dy = u - wl) is affine in DMA
    coordinates: per output row, ONE 3-dim DMA dumps contiguous
    41*56-element runs per partition to DRAM T with a per-partition
    (TROW-1) diagonal stride; dyp-period 56 == run width makes the dst
    fold to full-run descriptors.  A second DMA gathers displacement-
    major Z[w, dx*41+dy] (packed).
  - PE transposes flip Z into Zt[d, w]; Activation drains them so the
    GPSIMD-queued final store writes contiguous 512B w-runs.
  - Queues: SP = shear DMAs, Activation = transpose drains, DVE = PSUM
    drains, GPSIMD/SWDGE = output stores; the hq loop is software-
    pipelined so rect matmuls overlap the previous octet's shear.
"""

import ml_dtypes
import numpy as np

import concourse.bass as bass
import concourse.mybir as mybir
import concourse.tile as tile
from concourse.vector_clock import ScopedClock

F32 = mybir.dt.float32
F32R = mybir.dt.float32r
BF16 = mybir.dt.bfloat16

# ---------------------------------------------------------------------------
# Toolchain patches: this walrus build allows at most ONE sync-wait per
# instruction. (a) split the final TileContext drain's waits; (b) split any
# other multi-wait instruction at the BIR-JSON level before compilation.
# ---------------------------------------------------------------------------


def _patched_drain_and_barrier(self, tick_clock, wait_clock):
    drain_inst = self.nc.sync.drain()
    wait_clock.add_sem_waits(
        drain_inst.ins, ScopedClock({None: tick_clock.global_clock})
    )
    si = drain_inst.ins.sync_info
    if si is not None and len(si.on_wait) > 1:
        waits = list(si.on_wait)
        drain_inst.ins.sync_info = mybir.SyncInfo(
            on_wait=[waits[0]], on_update=list(si.on_update)
        )
        for w in waits[1:]:
            nop = self.nc.sync.nop(nofuse=True, hint="split_drain_wait")
            nop.ins.sync_info = mybir.SyncInfo(on_wait=[w], on_update=[])

    self.nc.all_engine_barrier()
    assert self.sems is not None
    popped = self.nc._tile_sem_poison_stack.pop()
    assert popped is self._sem_poison
    self.nc.clear_and_free_semaphores(list(self.sems.allocated().values()))
    self.nc.all_engine_barrier()


tile.TileContext._drain_and_barrier = _patched_drain_and_barrier

import orjson as _orjson
import concourse.bass_utils as _bass_utils


def _split_multi_waits_json(bir_json: bytes) -> bytes:
    bir = _orjson.loads(bir_json)
    counter = [0]
    changed = False
    for fn in bir.get("functions", []):
        for bb in fn.get("blocks", []) or []:
            insts = bb.get("instructions")
            if insts is None:
                continue
            new_insts = []
            for ins in insts:
                si = ins.get("sync_info")
                if si and len(si.get("on_wait") or []) > 1:
                    waits = si["on_wait"]
                    for w in waits[:-1]:
                        counter[0] += 1
                        new_insts.append({
                            "name": f"I-wsplit-{counter[0]}",
                            "opcode": "NoOp",
                            "engine": ins["engine"],
                            "ins": [],
                            "outs": [],
                            "sync_info": {"on_wait": [w], "on_update": []},
                        })
                    si["on_wait"] = [waits[-1]]
                    changed = True
                new_insts.append(ins)
            bb["instructions"] = new_insts
    if not changed:
        return bir_json
    return _orjson.dumps(bir)


_orig_compile_bir_kernel = _bass_utils.compile_bir_kernel


def _patched_compile_bir_kernel(bir_json, tmpdir, neff_name="file.neff"):
    return _orig_compile_bir_kernel(
        _split_multi_waits_json(bir_json), tmpdir, neff_name
    )


if getattr(_bass_utils.compile_bir_kernel, "__name__", "") != "_patched_compile_bir_kernel":
    _bass_utils.compile_bir_kernel = _patched_compile_bir_kernel
    try:
        import concourse.bass2jax as _bass2jax

        _bass2jax.compile_bir_kernel = _patched_compile_bir_kernel
    except Exception:
        pass

# ---------------------------------------------------------------------------
# Problem constants (hardcoded; kernel.py must be self-contained)
# ---------------------------------------------------------------------------
B, C, H, W = 4, 256, 64, 128
MD = 20
ND = 2 * MD + 1            # 41 displacements per axis
D2 = ND * ND               # 1681
HH = H // 2                # 32 h rows per core
SLABH = HH + 2 * MD        # 72
SLABW = W + 2 * MD         # 168
SROW = 2 * SLABH * SLABW   # slab free-dim row length per partition (24192)
CT = 16                    # col-tile width (pixels)
UW = CT + 2 * MD           # 56  u-window per col-tile
AH = 128 // CT             # 8 h-rows per weight rect (M = AH*CT = 128)
NJ = W // CT               # 8 w-tiles
SH = AH + 2 * MD           # 48  s-rows per rect
SR = SH * UW               # 2688 columns of S per rect per partition
DYP = UW                   # dyp stride == u-run width -> dump dst folds
TROW = ND * DYP + CT - 1   # +15 tail so spill stays off next partition
TST = 128 * TROW           # T elements per output row
NFULL = D2 // 128          # 13 full transpose chunks
NPART = D2 - NFULL * 128   # 17 leftover displacement channels
# s-row chunking for PSUM banks (N <= 512 fp32 per matmul)
S_CHUNKS = [(0, 9), (9, 9), (18, 9), (27, 9), (36, 9), (45, 3)]


def _build_nc():
    nc = bass.Bass()
    xpad = nc.declare_dram_parameter("xpad", [C, SLABH, SLABW], BF16, isOutput=False)
    out = nc.declare_dram_parameter("out", [D2, HH, W], F32, isOutput=True)
    T = nc.dram_tensor("T", [HH, 128, TROW], BF16)

    with tile.TileContext(nc) as tc:
        with (
            tc.tile_pool(name="slab", bufs=1) as slab_pool,
            tc.tile_pool(name="ident", bufs=1) as ident_pool,
            tc.tile_pool(name="ssb", bufs=2) as spool,
            tc.tile_pool(name="wt", bufs=3) as wpool,
            tc.tile_pool(name="z", bufs=4) as zpool,
            tc.tile_pool(name="zt", bufs=4) as ztpool,
            tc.tile_pool(name="pchunk", bufs=5, space=bass.MemorySpace.PSUM) as pchunk,
            tc.tile_pool(name="ptr", bufs=3, space=bass.MemorySpace.PSUM) as ptr,
        ):
            # persistent padded input slab: [c-partition, (chalf, hh, ww)]
            slab = slab_pool.tile([128, 2, SLABH, SLABW], BF16)
            # xpad[c, hh, ww] -> slab[c % 128, c // 128, hh, ww]
            nc.sync.dma_start(
                slab[:],
                xpad[:].rearrange("(ch p) hh ww -> p ch hh ww", ch=2),
            )

            # identity for PE transpose (bf16 to match the sheared S dtype)
            ident_i = ident_pool.tile([128, 128], mybir.dt.int32)
            nc.gpsimd.iota(ident_i[:], pattern=[[1, 128]], base=0,
                           channel_multiplier=-1)
            ident = ident_pool.tile([128, 128], BF16)
            nc.vector.tensor_scalar(ident[:], ident_i[:], 0, None,
                                    mybir.AluOpType.is_equal)

            slab_t = slab[:].tensor
            assert isinstance(slab[:].offset, int) and slab[:].offset == 0

            import os
            n_st = int(os.environ.get("KERNEL_ST_LIMIT", HH))
            n_hq = max(1, n_st // AH)

            def matmul_phase(hq):
                # ---- rect matmuls: S[(rh, wl), (j, s, u)], M = 128 ----
                # all NJ w-tiles share one big tile so the shear dump is a
                # single 3-dim DMA (j = free-dim stride SR).
                s_big = spool.tile([128, NJ * SR], BF16)
                all_copies = []
                for j in range(NJ):
                    # contiguous weight gather: AH h-rows x CT w pixels per ch
                    wts = wpool.tile([128, 2, 128], BF16)
                    for ch in range(2):
                        nc.vector.tensor_copy(
                            wts[:, ch, :].rearrange("p (a c) -> p a c", a=AH),
                            bass.AP(
                                slab_t,
                                ch * (SLABH * SLABW) + (AH * hq + MD) * SLABW
                                + CT * j + MD,
                                [[SROW, 128], [SLABW, AH], [1, CT]],
                            ),
                        )
                    for s0, ns in S_CHUNKS:
                        ps = pchunk.tile([128, 9 * UW], F32, tag="ps")
                        n = ns * UW
                        for ch in range(2):
                            rhs = bass.AP(
                                slab_t,
                                ch * (SLABH * SLABW) + (AH * hq + s0) * SLABW
                                + CT * j,
                                [[SROW, 128], [SLABW, ns], [1, UW]],
                            )
                            nc.tensor.matmul(
                                ps[:, 0:n], wts[:, ch, :], rhs,
                                start=(ch == 0), stop=(ch == 1),
                            )
                        all_copies.append(
                            nc.vector.tensor_copy(
                                s_big[:, j * SR + s0 * UW:j * SR + s0 * UW + n],
                                ps[:, 0:n],
                            )
                        )
                return s_big, all_copies

            def row_phase(hq, s_big, all_copies):
                s_t2 = s_big[:].tensor
                s_off2 = s_big[:].offset
                for rh in range(AH):
                    st = AH * hq + rh
                    # ---- shear hop 1: one DMA. src partition CT*rh + wl,
                    # j via free-dim stride SR; dst T[st, CT*j+wl, ...] with
                    # per-partition -1 shift so dyp = dy + CT - 1 uniformly.
                    src = bass.AP(
                        s_t2,
                        s_off2 + CT * rh * (NJ * SR) + rh * UW,
                        [[NJ * SR, CT], [SR, NJ], [1, ND * UW]],
                    )
                    dst = bass.AP(
                        T,
                        st * TST + CT - 1,
                        [[TROW - 1, CT], [CT * TROW, NJ], [1, ND * UW]],
                    )
                    d = nc.sync.dma_start(dst, src)
                    for cp in all_copies:
                        tile.add_dep_helper(d.ins, cp.ins, reason="S->shear")

                    # ---- shear hop 2: packed displacement-major gather ----
                    z = zpool.tile([128, D2], BF16)
                    rl = nc.sync.dma_start(
                        z[:],
                        bass.AP(T, st * TST + CT - 1,
                                [[TROW, 128], [DYP, ND], [1, ND]]),
                    )
                    tile.add_dep_helper(rl.ins, d.ins, reason="dump->reload")

                    # ---- transpose to Zt[d, (chunk, w)] and store ----
                    zt = ztpool.tile([128, (NFULL + 1) * 128], F32)
                    for cix in range(NFULL + 1):
                        ncols = 128 if cix < NFULL else NPART
                        tp = ptr.tile([128, 128], BF16, tag="tp")
                        tr = nc.tensor.transpose(
                            tp[0:ncols, :], z[:, 128 * cix:128 * cix + ncols],
                            ident[:],
                        )
                        tile.add_dep_helper(tr.ins, rl.ins, reason="shear->tr")
                        nc.scalar.copy(
                            zt[0:ncols, 128 * cix:128 * cix + 128],
                            tp[0:ncols, :],
                        )

                    zt_t = zt[:].tensor
                    zt_off = zt[:].offset
                    # full chunks: d = 128*c + dd
                    nc.gpsimd.dma_start(
                        bass.AP(out, st * W,
                                [[HH * W, 128], [128 * HH * W, NFULL], [1, W]]),
                        bass.AP(zt_t, zt_off,
                                [[(NFULL + 1) * 128, 128], [128, NFULL], [1, W]]),
                    )
                    nc.gpsimd.dma_start(
                        bass.AP(out, NFULL * 128 * HH * W + st * W,
                                [[HH * W, NPART], [1, W]]),
                        bass.AP(zt_t, zt_off + NFULL * 128,
                                [[(NFULL + 1) * 128, NPART], [1, W]]),
                    )

            # software pipeline: hq's matmuls overlap hq-1's shear/transpose
            prev = None
            for hq in range(n_hq):
                state = matmul_phase(hq)
                if prev is not None:
                    row_phase(prev[0], prev[1], prev[2])
                prev = (hq, state[0], state[1])
            row_phase(prev[0], prev[1], prev[2])
    return nc


_CACHE = {}


def _get_nc():
    if "nc" not in _CACHE:
        _CACHE["nc"] = _build_nc()
    return _CACHE["nc"]


def kernel(x_1: np.ndarray, _trace: bool = False) -> np.ndarray:
    from concourse.bass_utils import run_bass_kernel_spmd

    x_1 = np.ascontiguousarray(x_1, dtype=np.float32)
    assert x_1.shape == (B, C, H, W)
    nc = _get_nc()

    in_maps = []
    for core in range(8):
        b, half = core // 2, core % 2
        H0 = HH * half
        slab = np.zeros((C, SLABH, SLABW), ml_dtypes.bfloat16)
        h_lo = max(0, H0 - MD)
        h_hi = min(H, H0 + HH + MD)
        slab[:, (h_lo - H0 + MD):(h_hi - H0 + MD), MD:MD + W] = x_1[
            b, :, h_lo:h_hi, :
        ].astype(ml_dtypes.bfloat16)
        in_maps.append({"xpad": slab})

    res = run_bass_kernel_spmd(nc, in_maps, core_ids=list(range(8)), trace=_trace)
    _CACHE["last_results"] = res
    out = np.empty((B, D2, H, W), np.float32)
    for core in range(8):
        b, half = core // 2, core % 2
        out[b, :, HH * half:HH * (half + 1), :] = res.results[core]["out"]
    return out


if __name__ == "__main__":
    x = np.random.randn(B, C, H, W).astype(np.float32)
    y = kernel(x)
    print("kernel output shape:", y.shape)



# revision 2
# speedup vs baseline: 1.0084x; 1.0084x over previous
"""Trainium2 Bass kernel for a 2D correlation layer (v2).

out[b, dx*41+dy, h, w] = sum_c x[b,c,h,w] * xpad[b,c,h+dx,w+dy]
with x of shape (4, 256, 64, 128), 41x41 displacements (max_disp 20).

Strategy (8 NeuronCores, SPMD): core = b*2 + half computes rows
[32*half, 32*half+32).  Per core:

  - Persistent bf16 slab [128, 2, 72, 168] (c%128 partition, c//128 plane,
    padded h, padded w), loaded in 4 row-band DMAs for early start.
  - Rects of AH=16 h-rows x CT=8 w-cols (M=128 pixels, p = 8a + c).
    Per rect (hq in {0,1}, j in [0,16)): 6 PSUM chunks (<=480 cols),
    each = 2 bf16 matmuls (K=128 per channel half) over the (s,u)
    window [56 x 48].  Weights come straight from the slab (pixel block),
    moving operand is the slab window -- no gather copies.
  - Drains: PSUM fp32 -> S tile bf16, alternating Activation / DVE.
    S tiles are per (hq, jhalf): [128, 8, 2688], pool bufs=3.
  - Fused shear-store: per (hq, a, jhalf) ONE DMA writes the final
    displacement-major-per-pixel layout directly to DRAM:
      src  = S[8a:8a+8, :, 48a : 48a+1968]          (pure tile slice)
      dst  = out48[st, w, :] with per-c shift -c done on the DRAM side
             via c-stride (SLOT-1); slot pad absorbs the bleed.
    out48[st, w, dx*48+dy] (dy<41 valid) -- host strips pad, transposes.
  No DRAM round-trip, no PE transposes, no output-layout pass: the shear
  IS the store.  DMA runs are 3936 B, so descriptors stay full-rate.
"""

import ml_dtypes
import numpy as np

import concourse.bass as bass
import concourse.mybir as mybir
import concourse.tile as tile
from concourse.vector_clock import ScopedClock

F32 = mybir.dt.float32
BF16 = mybir.dt.bfloat16
FP8 = mybir.dt.float8e4

# ---------------------------------------------------------------------------
# Toolchain patches (same as baseline): this walrus build allows at most ONE
# sync-wait per instruction; split multi-wait instructions.
# ---------------------------------------------------------------------------


def _patched_drain_and_barrier(self, tick_clock, wait_clock):
    drain_inst = self.nc.sync.drain()
    wait_clock.add_sem_waits(
        drain_inst.ins, ScopedClock({None: tick_clock.global_clock})
    )
    si = drain_inst.ins.sync_info
    if si is not None and len(si.on_wait) > 1:
        waits = list(si.on_wait)
        drain_inst.ins.sync_info = mybir.SyncInfo(
            on_wait=[waits[0]], on_update=list(si.on_update)
        )
        for w in waits[1:]:
            nop = self.nc.sync.nop(nofuse=True, hint="split_drain_wait")
            nop.ins.sync_info = mybir.SyncInfo(on_wait=[w], on_update=[])

    self.nc.all_engine_barrier()
    assert self.sems is not None
    popped = self.nc._tile_sem_poison_stack.pop()
    assert popped is self._sem_poison
    self.nc.clear_and_free_semaphores(list(self.sems.allocated().values()))
    self.nc.all_engine_barrier()


tile.TileContext._drain_and_barrier = _patched_drain_and_barrier

import orjson as _orjson
import concourse.bass_utils as _bass_utils


def _split_multi_waits_json(bir_json: bytes) -> bytes:
    bir = _orjson.loads(bir_json)
    counter = [0]
    changed = False
    for fn in bir.get("functions", []):
        for bb in fn.get("blocks", []) or []:
            insts = bb.get("instructions")
            if insts is None:
                continue
            new_insts = []
            for ins in insts:
                si = ins.get("sync_info")
                if si and len(si.get("on_wait") or []) > 1:
                    waits = si["on_wait"]
                    for w in waits[:-1]:
                        counter[0] += 1
                        new_insts.append({
                            "name": f"I-wsplit-{counter[0]}",
                            "opcode": "NoOp",
                            "engine": ins["engine"],
                            "ins": [],
                            "outs": [],
                            "sync_info": {"on_wait": [w], "on_update": []},
                        })
                    si["on_wait"] = [waits[-1]]
                    changed = True
                new_insts.append(ins)
            bb["instructions"] = new_insts
    if not changed:
        return bir_json
    return _orjson.dumps(bir)


_orig_compile_bir_kernel = _bass_utils.compile_bir_kernel


def _patched_compile_bir_kernel(bir_json, tmpdir, neff_name="file.neff"):
    return _orig_compile_bir_kernel(
        _split_multi_waits_json(bir_json), tmpdir, neff_name
    )


if getattr(_bass_utils.compile_bir_kernel, "__name__", "") != "_patched_compile_bir_kernel":
    _bass_utils.compile_bir_kernel = _patched_compile_bir_kernel
    try:
        import concourse.bass2jax as _bass2jax

        _bass2jax.compile_bir_kernel = _patched_compile_bir_kernel
    except Exception:
        pass

# ---------------------------------------------------------------------------
# Problem constants
# ---------------------------------------------------------------------------
B, C, H, W = 4, 256, 64, 128
MD = 20
ND = 2 * MD + 1            # 41
D2 = ND * ND               # 1681
HH = H // 2                # 32 rows per core
SLABH = HH + 2 * MD        # 72
SLABW = W + 2 * MD         # 168
SROW = 2 * SLABH * SLABW   # slab free elems per partition (24192)

AH = 16                    # rect h-rows
CT = 8                     # rect w-cols  (M = AH*CT = 128)
UW = CT + 2 * MD           # 48: u-window per rect
SH = AH + 2 * MD           # 56: s-window per rect
SR = SH * UW               # 2688 S columns per rect
NJ = W // CT               # 16 w-tiles (j)
NHQ = HH // AH             # 2 h-groups (hq)
JH = NJ // 2               # 8 j per S tile (jhalf)
RUN = ND * UW              # 1968: elems per (pixel) shear run
SLOT = RUN + 8             # 1976: out48 slot per pixel (pad absorbs shear bleed)
SRL = JH * SR              # S tile free elems per partition (21504)

# s-chunks per rect: (s0, ns) with ns*UW <= 512 psum fp32 cols
S_CHUNKS = [(0, 10), (10, 10), (20, 10), (30, 10), (40, 10), (50, 6)]


def _build_nc():
    nc = bass.Bass()
    slab_d = nc.declare_dram_parameter("slab", [128, 2, 2, SLABH, SLABW], FP8,
                                       isOutput=False)
    wts_d = nc.declare_dram_parameter("wts", [128, 2, NHQ * NJ, 2, 128], FP8,
                                      isOutput=False)
    out48 = nc.declare_dram_parameter("out48", [HH, W, SLOT], BF16,
                                      isOutput=True)

    with tile.TileContext(nc) as tc:
        with (
            tc.tile_pool(name="slab", bufs=1) as slab_pool,
            tc.tile_pool(name="sq", bufs=2) as qpool,
            tc.tile_pool(name="ps", bufs=4, space=bass.MemorySpace.PSUM) as ppool,
        ):
            # dims: [k, variant (x8/r8), ch-ktile, h, w] / [k, variant, rect, ch, m]
            slab = slab_pool.tile([128, 2, 2, SLABH, SLABW], FP8)
            wts = slab_pool.tile([128, 2, NHQ * NJ, 2, 128], FP8)

            # PE p-state warmup: the PE runs at reduced clock for the first
            # 3us of continuous busy.  Burn that ramp on zero matmuls while
            # the first loads are in flight, so real matmuls start hot.
            warm = slab_pool.tile([128, 2, 512], FP8)
            nc.vector.memset(warm[:], 0.0)
            for _ in range(24):
                wps = ppool.tile([128, 2, 512], F32, tag="ps")
                nc.tensor.matmul(
                    wps[:, 0, :], warm[:, :, 0:128], warm[:],
                    start=True, stop=True,
                    perf_mode=mybir.MatmulPerfMode.DoubleRow,
                )

            # interleaved loads: per-rect-range weights + 8-row slab bands
            # (both fp8 variants) so the chunk-major sweep starts ~5us in.
            def load_wts(hq, v, lo=0, hi=NJ):
                r0 = hq * NJ
                nc.sync.dma_start(wts[:, v, r0 + lo:r0 + hi, :, :],
                                  wts_d[:, v, r0 + lo:r0 + hi, :, :])

            def load_band(k, v):
                r0, r1 = 8 * k, min(8 * k + 8, SLABH)
                nc.sync.dma_start(slab[:, v, :, r0:r1, :],
                                  slab_d[:, v, :, r0:r1, :])

            def gather_wts(hq, j):
                # on-chip weight gather (Pool is otherwise idle): pixel block
                # rows AH*hq+MD+a, cols CT*j+MD+c out of the slab
                r = hq * NJ + j
                for v in range(2):
                    for ch in range(2):
                        dst = wts[:, v, r, ch, :].rearrange(
                            "p (a c) -> p a c", a=AH)
                        src = slab[:, v, ch,
                                   AH * hq + MD:AH * hq + MD + AH,
                                   CT * j + MD:CT * j + MD + CT]
                        nc.gpsimd.tensor_copy(dst, src)

            # G1 (hq1, j<8) runs first: its rows/weights lead (weights via
            # DMA -- the on-chip gather would gate startup on bands 4-6).
            # G0/G2's weights are gathered on-chip from the slab instead.
            load_wts(1, 0, 0, 2)
            load_wts(1, 1, 0, 2)
            load_band(2, 0)
            load_band(2, 1)
            load_band(3, 0)
            load_band(3, 1)
            load_wts(1, 0, 2, JH)
            load_wts(1, 1, 2, JH)
            for k in (4, 5, 6, 7, 8):
                load_band(k, 0)
                load_band(k, 1)
            load_band(0, 0)
            load_band(0, 1)
            load_band(1, 0)
            load_band(1, 1)
            for j in range(NJ):
                gather_wts(0, j)
            for j in range(JH, NJ):
                gather_wts(1, j)

            drain_acc = [0.0, 0.0]   # est. busy ns: [Act, DVE]

            def drain(dst, src, n):
                cost_act = (n + 172) * 0.8333
                cost_dve = (n + 120) * 1.0417
                if drain_acc[0] + cost_act <= drain_acc[1] + cost_dve:
                    drain_acc[0] += cost_act
                    nc.scalar.copy(dst, src)
                else:
                    drain_acc[1] += cost_dve
                    nc.vector.tensor_copy(dst, src)

            def shear_dma(S, hq, woff, nj, a):
                st = AH * hq + a
                src = S[CT * a:CT * a + CT, :, UW * a:UW * a + RUN]
                dst = bass.AP(
                    out48,
                    st * (W * SLOT) + (woff * CT) * SLOT,
                    [[SLOT - 1, CT], [CT * SLOT, nj], [1, RUN]],
                )
                nc.sync.dma_start(dst, src)

            def do_group(S, hq, woff, nj):
                """One (hq, j-range) group, chunk-major: for each s-chunk row,
                all rects' matmul triples + drains; shear DMAs fire in two
                batches (a<10 needs only chunks s0<=40)."""
                for ci, (s0, ns) in enumerate(S_CHUNKS):
                    n = ns * UW
                    for jq in range(nj // 2):
                        # 2 j-chunks share one 2-bank PSUM tile; ONE strided
                        # drain moves both (amortizes the copy init)
                        ps = ppool.tile([128, 2, 512], F32, tag="ps")
                        for jj in range(2):
                            j = woff + 2 * jq + jj
                            # error-feedback fp8: x ~ x8 + r8; keep the three
                            # O(x8*x8), O(x8*r8), O(r8*x8) terms, each a K=256
                            # DoubleRow matmul at 0.5 cyc/col
                            for mi, (wv, sv) in enumerate(
                                    ((0, 0), (0, 1), (1, 0))):
                                lhsT = wts[:, wv, hq * NJ + j, :, :]
                                rhs = slab[:, sv, :,
                                           AH * hq + s0:AH * hq + s0 + ns,
                                           CT * j:CT * j + UW]
                                nc.tensor.matmul(
                                    ps[:, jj, 0:n], lhsT, rhs,
                                    start=(mi == 0), stop=(mi == 2),
                                    perf_mode=mybir.MatmulPerfMode.DoubleRow,
                                )
                        drain(S[:, 2 * jq:2 * jq + 2, s0 * UW:s0 * UW + n],
                              ps[:, :, 0:n], 2 * n)
                    if ci == len(S_CHUNKS) - 2:
                        # chunks s0<=40 done: rows a<10 fully drained
                        for a in range(10):
                            shear_dma(S, hq, woff, nj, a)
                for a in range(10, AH):
                    shear_dma(S, hq, woff, nj, a)

            # 4 groups of 8 j, ordered hq1-jh0, hq0-jh0, hq0-jh1, hq1-jh1:
            # drain windows pace ~13.5us apart so each group's shear slots
            # into the DMA device while the next group drains.  Two rotating
            # S buffers (tag Sh); reuse only after the old group's shears.
            for gi, (hq, woff) in enumerate([(1, 0), (0, 0), (0, JH), (1, JH)]):
                S = qpool.tile([128, JH, SR], BF16, name=f"S{gi}", tag="Sh")
                do_group(S, hq, woff, JH)
    return nc


_CACHE = {}


def _get_nc():
    if "nc" not in _CACHE:
        _CACHE["nc"] = _build_nc()
    return _CACHE["nc"]


def kernel(x_1: np.ndarray, _trace: bool = False) -> np.ndarray:
    from concourse.bass_utils import run_bass_kernel_spmd

    x_1 = np.ascontiguousarray(x_1, dtype=np.float32)
    assert x_1.shape == (B, C, H, W)
    nc = _get_nc()

    in_maps = []
    for core in range(8):
        b, half = core // 2, core % 2
        H0 = HH * half
        f8 = ml_dtypes.float8_e4m3
        pad = np.zeros((C, SLABH, SLABW), np.float32)
        h_lo = max(0, H0 - MD)
        h_hi = min(H, H0 + HH + MD)
        pad[:, (h_lo - H0 + MD):(h_hi - H0 + MD), MD:MD + W] = x_1[
            b, :, h_lo:h_hi, :
        ]
        x8 = pad.astype(f8)
        r8 = (pad - x8.astype(np.float32)).astype(f8)
        # [v, c, hh, ww] -> [c%128 partition, v, c//128 ktile, hh, ww]
        vs = np.stack([x8, r8], axis=0)                  # [2, C, 72, 168]
        slab = np.ascontiguousarray(
            vs.reshape(2, 2, 128, SLABH, SLABW).transpose(2, 0, 1, 3, 4)
        )
        # weights: wts[k, v, hq*NJ + j, ch, a*CT + c]
        px = slab[:, :, :, MD:MD + HH, MD:MD + W]        # [128, 2, 2, 32, 128]
        w7 = px.reshape(128, 2, 2, NHQ, AH, NJ, CT)
        wts_np = np.ascontiguousarray(
            w7.transpose(0, 1, 3, 5, 2, 4, 6).reshape(128, 2, NHQ * NJ, 2, 128)
        )
        in_maps.append({"slab": slab, "wts": wts_np})

    res = run_bass_kernel_spmd(nc, in_maps, core_ids=list(range(8)), trace=_trace)
    _CACHE["last_results"] = res
    out = np.empty((B, D2, H, W), np.float32)
    for core in range(8):
        b, half = core // 2, core % 2
        o = np.asarray(res.results[core]["out48"])  # [HH, W, SLOT] bf16
        o = o[:, :, :RUN].astype(np.float32).reshape(HH, W, ND, UW)[:, :, :, :ND]
        out[b, :, HH * half:HH * (half + 1), :] = (
            o.transpose(2, 3, 0, 1).reshape(D2, HH, W)
        )
    return out


if __name__ == "__main__":
    x = np.random.randn(B, C, H, W).astype(np.float32)
    y = kernel(x)
    print("kernel output shape:", y.shape)


# revision 3
# speedup vs baseline: 1.0145x; 1.0061x over previous
"""Trainium2 Bass kernel for a 2D correlation layer (v2).

out[b, dx*41+dy, h, w] = sum_c x[b,c,h,w] * xpad[b,c,h+dx,w+dy]
with x of shape (4, 256, 64, 128), 41x41 displacements (max_disp 20).

Strategy (8 NeuronCores, SPMD): core = b*2 + half computes rows
[32*half, 32*half+32).  Per core:

  - Persistent bf16 slab [128, 2, 72, 168] (c%128 partition, c//128 plane,
    padded h, padded w), loaded in 4 row-band DMAs for early start.
  - Rects of AH=16 h-rows x CT=8 w-cols (M=128 pixels, p = 8a + c).
    Per rect (hq in {0,1}, j in [0,16)): 6 PSUM chunks (<=480 cols),
    each = 2 bf16 matmuls (K=128 per channel half) over the (s,u)
    window [56 x 48].  Weights come straight from the slab (pixel block),
    moving operand is the slab window -- no gather copies.
  - Drains: PSUM fp32 -> S tile bf16, alternating Activation / DVE.
    S tiles are per (hq, jhalf): [128, 8, 2688], pool bufs=3.
  - Fused shear-store: per (hq, a, jhalf) ONE DMA writes the final
    displacement-major-per-pixel layout directly to DRAM:
      src  = S[8a:8a+8, :, 48a : 48a+1968]          (pure tile slice)
      dst  = out48[st, w, :] with per-c shift -c done on the DRAM side
             via c-stride (SLOT-1); slot pad absorbs the bleed.
    out48[st, w, dx*48+dy] (dy<41 valid) -- host strips pad, transposes.
  No DRAM round-trip, no PE transposes, no output-layout pass: the shear
  IS the store.  DMA runs are 3936 B, so descriptors stay full-rate.
"""

import ml_dtypes
import numpy as np

import concourse.bass as bass
import concourse.mybir as mybir
import concourse.tile as tile
from concourse.vector_clock import ScopedClock

F32 = mybir.dt.float32
BF16 = mybir.dt.bfloat16
FP8 = mybir.dt.float8e4

# ---------------------------------------------------------------------------
# Toolchain patches (same as baseline): this walrus build allows at most ONE
# sync-wait per instruction; split multi-wait instructions.
# ---------------------------------------------------------------------------


def _patched_drain_and_barrier(self, tick_clock, wait_clock):
    drain_inst = self.nc.sync.drain()
    wait_clock.add_sem_waits(
        drain_inst.ins, ScopedClock({None: tick_clock.global_clock})
    )
    si = drain_inst.ins.sync_info
    if si is not None and len(si.on_wait) > 1:
        waits = list(si.on_wait)
        drain_inst.ins.sync_info = mybir.SyncInfo(
            on_wait=[waits[0]], on_update=list(si.on_update)
        )
        for w in waits[1:]:
            nop = self.nc.sync.nop(nofuse=True, hint="split_drain_wait")
            nop.ins.sync_info = mybir.SyncInfo(on_wait=[w], on_update=[])

    self.nc.all_engine_barrier()
    assert self.sems is not None
    popped = self.nc._tile_sem_poison_stack.pop()
    assert popped is self._sem_poison
    self.nc.clear_and_free_semaphores(list(self.sems.allocated().values()))
    self.nc.all_engine_barrier()


tile.TileContext._drain_and_barrier = _patched_drain_and_barrier

import orjson as _orjson
import concourse.bass_utils as _bass_utils


def _split_multi_waits_json(bir_json: bytes) -> bytes:
    bir = _orjson.loads(bir_json)
    counter = [0]
    changed = False
    for fn in bir.get("functions", []):
        for bb in fn.get("blocks", []) or []:
            insts = bb.get("instructions")
            if insts is None:
                continue
            new_insts = []
            for ins in insts:
                si = ins.get("sync_info")
                if si and len(si.get("on_wait") or []) > 1:
                    waits = si["on_wait"]
                    for w in waits[:-1]:
                        counter[0] += 1
                        new_insts.append({
                            "name": f"I-wsplit-{counter[0]}",
                            "opcode": "NoOp",
                            "engine": ins["engine"],
                            "ins": [],
                            "outs": [],
                            "sync_info": {"on_wait": [w], "on_update": []},
                        })
                    si["on_wait"] = [waits[-1]]
                    changed = True
                new_insts.append(ins)
            bb["instructions"] = new_insts
    if not changed:
        return bir_json
    return _orjson.dumps(bir)


_orig_compile_bir_kernel = _bass_utils.compile_bir_kernel


def _patched_compile_bir_kernel(bir_json, tmpdir, neff_name="file.neff"):
    return _orig_compile_bir_kernel(
        _split_multi_waits_json(bir_json), tmpdir, neff_name
    )


if getattr(_bass_utils.compile_bir_kernel, "__name__", "") != "_patched_compile_bir_kernel":
    _bass_utils.compile_bir_kernel = _patched_compile_bir_kernel
    try:
        import concourse.bass2jax as _bass2jax

        _bass2jax.compile_bir_kernel = _patched_compile_bir_kernel
    except Exception:
        pass

# ---------------------------------------------------------------------------
# Problem constants
# ---------------------------------------------------------------------------
B, C, H, W = 4, 256, 64, 128
MD = 20
ND = 2 * MD + 1            # 41
D2 = ND * ND               # 1681
HH = H // 2                # 32 rows per core
SLABH = HH + 2 * MD        # 72
SLABW = W + 2 * MD         # 168
SROW = 2 * SLABH * SLABW   # slab free elems per partition (24192)

AH = 16                    # rect h-rows
CT = 8                     # rect w-cols  (M = AH*CT = 128)
UW = CT + 2 * MD           # 48: u-window per rect
SH = AH + 2 * MD           # 56: s-window per rect
SR = SH * UW               # 2688 S columns per rect
NJ = W // CT               # 16 w-tiles (j)
NHQ = HH // AH             # 2 h-groups (hq)
JH = NJ // 2               # 8 j per S tile (jhalf)
RUN = ND * UW              # 1968: elems per (pixel) shear run
SLOT = RUN + 8             # 1976: out48 slot per pixel (pad absorbs shear bleed)
SLOT2 = ND * 56 + 16       # 2312: slot for the 8x16-rect (CT=16) quadrant
SRL = JH * SR              # S tile free elems per partition (21504)

# s-chunks per rect: (s0, ns) with ns*UW <= 512 psum fp32 cols
S_CHUNKS = [(0, 10), (10, 10), (20, 10), (30, 10), (40, 10), (50, 6)]


def _build_nc():
    nc = bass.Bass()
    slab_d = nc.declare_dram_parameter("slab", [128, 2, 2, SLABH, SLABW], FP8,
                                       isOutput=False)
    wts_d = nc.declare_dram_parameter("wts", [128, 2, NHQ * NJ, 2, 128], FP8,
                                      isOutput=False)
    out48 = nc.declare_dram_parameter("out48", [HH, W, SLOT], BF16,
                                      isOutput=True)
    # rows 16-32 x cols 64-128 use 8x16 rects (AH=8, CT=16, UW=56): separate
    # layout with 2296+16 slots, halving the final shear-DMA count
    out56 = nc.declare_dram_parameter("out56", [AH, W - JH * CT, SLOT2],
                                      BF16, isOutput=True)

    with tile.TileContext(nc) as tc:
        with (
            tc.tile_pool(name="slab", bufs=1) as slab_pool,
            tc.tile_pool(name="sq", bufs=3) as qpool,
            tc.tile_pool(name="ps", bufs=4, space=bass.MemorySpace.PSUM) as ppool,
        ):
            # dims: [k, variant (x8/r8), ch-ktile, h, w] / [k, variant, rect, ch, m]
            slab = slab_pool.tile([128, 2, 2, SLABH, SLABW], FP8)
            wts = slab_pool.tile([128, 2, NHQ * NJ, 2, 128], FP8)

            # PE p-state warmup: the PE runs at reduced clock for the first
            # 3us of continuous busy.  Burn that ramp on zero matmuls while
            # the first loads are in flight, so real matmuls start hot.
            warm = slab_pool.tile([128, 2, 512], FP8)
            nc.vector.memset(warm[:], 0.0)
            for _ in range(24):
                wps = ppool.tile([128, 2, 512], F32, tag="ps")
                nc.tensor.matmul(
                    wps[:, 0, :], warm[:, :, 0:128], warm[:],
                    start=True, stop=True,
                    perf_mode=mybir.MatmulPerfMode.DoubleRow,
                )

            # interleaved loads: per-rect-range weights + 8-row slab bands
            # (both fp8 variants) so the chunk-major sweep starts ~5us in.
            def load_wts(hq, v, lo=0, hi=NJ):
                r0 = hq * NJ
                nc.sync.dma_start(wts[:, v, r0 + lo:r0 + hi, :, :],
                                  wts_d[:, v, r0 + lo:r0 + hi, :, :])

            def load_band(k, v):
                r0, r1 = 8 * k, min(8 * k + 8, SLABH)
                nc.sync.dma_start(slab[:, v, :, r0:r1, :],
                                  slab_d[:, v, :, r0:r1, :])

            def gather_wts(r, rows0, AHg, CTg, col0):
                # on-chip weight gather (Pool is otherwise idle): the rect's
                # pixel block rows rows0+MD+a, cols col0+MD+c from the slab
                for v in range(2):
                    for ch in range(2):
                        dst = wts[:, v, r, ch, :].rearrange(
                            "p (a c) -> p a c", a=AHg)
                        src = slab[:, v, ch,
                                   rows0 + MD:rows0 + MD + AHg,
                                   col0 + MD:col0 + MD + CTg]
                        nc.gpsimd.tensor_copy(dst, src)

            # G1 (hq1, j<8) runs first: its rows/weights lead (weights via
            # DMA -- the on-chip gather would gate startup on bands 4-6).
            # G0/G2's weights are gathered on-chip from the slab instead.
            load_wts(1, 0, 0, 2)
            load_wts(1, 1, 0, 2)
            load_band(2, 0)
            load_band(2, 1)
            load_band(3, 0)
            load_band(3, 1)
            load_wts(1, 0, 2, JH)
            load_wts(1, 1, 2, JH)
            for k in (4, 5, 6, 7, 8):
                load_band(k, 0)
                load_band(k, 1)
            load_band(0, 0)
            load_band(0, 1)
            load_band(1, 0)
            load_band(1, 1)
            for j in range(NJ):
                gather_wts(j, 0, AH, CT, CT * j)
            for g in range(2):
                for jt in range(4):
                    gather_wts(NJ + JH + 4 * g + jt, AH + 8 * g, 8, 16,
                               JH * CT + 16 * jt)

            drain_acc = [0.0, 0.0]   # est. busy ns: [Act, DVE]

            def drain(dst, src, n):
                cost_act = (n + 172) * 0.8333
                cost_dve = (n + 120) * 1.0417
                if drain_acc[0] + cost_act <= drain_acc[1] + cost_dve:
                    drain_acc[0] += cost_act
                    nc.scalar.copy(dst, src)
                else:
                    drain_acc[1] += cost_dve
                    nc.vector.tensor_copy(dst, src)

            def do_group(S, rows0, AHg, CTg, w0, ncols, wbase, chunks,
                         out_t, out_r0, out_w0, out_w, slot):
                """One group of 128-pixel rects (AHg x CTg), chunk-major:
                for each s-chunk row, all rects' matmul triples + pair
                drains; shear DMAs (one per rect-row a) fire in two batches
                (a < s0_last-40 only needs the earlier chunks)."""
                UWg = CTg + 2 * MD
                RUNg = ND * UWg
                njt = ncols // CTg

                def shear_dma(a):
                    src = S[CTg * a:CTg * a + CTg, :, UWg * a:UWg * a + RUNg]
                    dst = bass.AP(
                        out_t,
                        (rows0 + a - out_r0) * (out_w * slot)
                        + (w0 - out_w0) * slot,
                        [[slot - 1, CTg], [CTg * slot, njt], [1, RUNg]],
                    )
                    nc.sync.dma_start(dst, src)

                a_thr = chunks[-1][0] - 2 * MD
                for ci, (s0, ns) in enumerate(chunks):
                    n = ns * UWg
                    for jq in range(njt // 2):
                        # 2 rect-chunks share one 2-bank PSUM tile; ONE
                        # strided drain moves both (amortizes the copy init)
                        ps = ppool.tile([128, 2, 512], F32, tag="ps")
                        for jj in range(2):
                            jtl = 2 * jq + jj
                            # error-feedback fp8: x ~ x8 + r8; the three
                            # O(x8*x8), O(x8*r8), O(r8*x8) terms, each a
                            # K=256 DoubleRow matmul at 0.5 cyc/col
                            for mi, (wv, sv) in enumerate(
                                    ((0, 0), (0, 1), (1, 0))):
                                lhsT = wts[:, wv, wbase + jtl, :, :]
                                rhs = slab[:, sv, :,
                                           rows0 + s0:rows0 + s0 + ns,
                                           w0 + CTg * jtl:
                                           w0 + CTg * jtl + UWg]
                                nc.tensor.matmul(
                                    ps[:, jj, 0:n], lhsT, rhs,
                                    start=(mi == 0), stop=(mi == 2),
                                    perf_mode=mybir.MatmulPerfMode.DoubleRow,
                                )
                        drain(S[:, 2 * jq:2 * jq + 2, s0 * UWg:s0 * UWg + n],
                              ps[:, :, 0:n], 2 * n)
                    if ci == len(chunks) - 2:
                        for a in range(a_thr):
                            shear_dma(a)
                for a in range(a_thr, AHg):
                    shear_dma(a)

            # chunk rows per rect geometry (ns*UW <= 512 psum fp32 cols)
            CH8 = [(0, 10), (10, 10), (20, 10), (30, 10), (40, 10), (50, 6)]
            CH16 = [(0, 9), (9, 9), (18, 9), (27, 9), (36, 9), (45, 3)]

            # groups: three 16x8-rect blocks, then the rows16-32/cols64-128
            # quadrant as two 8-row blocks of 8x16 rects (8 shear DMAs each,
            # so the final shear tail is halved).  Two rotating S buffers.
            S0 = qpool.tile([128, JH, SR], BF16, tag="Sh")
            do_group(S0, AH, AH, CT, 0, JH * CT, NJ, CH8,
                     out48, 0, 0, W, SLOT)
            S1 = qpool.tile([128, JH, SR], BF16, tag="Sh")
            do_group(S1, 0, AH, CT, 0, JH * CT, 0, CH8,
                     out48, 0, 0, W, SLOT)
            S2 = qpool.tile([128, JH, SR], BF16, tag="Sh")
            do_group(S2, 0, AH, CT, JH * CT, JH * CT, JH, CH8,
                     out48, 0, 0, W, SLOT)
            S3 = qpool.tile([128, 4, SR], BF16, tag="Sh")
            do_group(S3, AH, 8, 16, JH * CT, JH * CT, NJ + JH, CH16,
                     out56, AH, JH * CT, JH * CT, SLOT2)
            S4 = qpool.tile([128, 4, SR], BF16, tag="Sh")
            do_group(S4, AH + 8, 8, 16, JH * CT, JH * CT, NJ + JH + 4, CH16,
                     out56, AH, JH * CT, JH * CT, SLOT2)
    return nc


_CACHE = {}


def _get_nc():
    if "nc" not in _CACHE:
        _CACHE["nc"] = _build_nc()
    return _CACHE["nc"]


def kernel(x_1: np.ndarray, _trace: bool = False) -> np.ndarray:
    from concourse.bass_utils import run_bass_kernel_spmd

    x_1 = np.ascontiguousarray(x_1, dtype=np.float32)
    assert x_1.shape == (B, C, H, W)
    nc = _get_nc()

    in_maps = []
    for core in range(8):
        b, half = core // 2, core % 2
        H0 = HH * half
        f8 = ml_dtypes.float8_e4m3
        pad = np.zeros((C, SLABH, SLABW), np.float32)
        h_lo = max(0, H0 - MD)
        h_hi = min(H, H0 + HH + MD)
        pad[:, (h_lo - H0 + MD):(h_hi - H0 + MD), MD:MD + W] = x_1[
            b, :, h_lo:h_hi, :
        ]
        x8 = pad.astype(f8)
        r8 = (pad - x8.astype(np.float32)).astype(f8)
        # [v, c, hh, ww] -> [c%128 partition, v, c//128 ktile, hh, ww]
        vs = np.stack([x8, r8], axis=0)                  # [2, C, 72, 168]
        slab = np.ascontiguousarray(
            vs.reshape(2, 2, 128, SLABH, SLABW).transpose(2, 0, 1, 3, 4)
        )
        # weights: wts[k, v, hq*NJ + j, ch, a*CT + c]
        px = slab[:, :, :, MD:MD + HH, MD:MD + W]        # [128, 2, 2, 32, 128]
        w7 = px.reshape(128, 2, 2, NHQ, AH, NJ, CT)
        wts_np = np.ascontiguousarray(
            w7.transpose(0, 1, 3, 5, 2, 4, 6).reshape(128, 2, NHQ * NJ, 2, 128)
        )
        in_maps.append({"slab": slab, "wts": wts_np})

    res = run_bass_kernel_spmd(nc, in_maps, core_ids=list(range(8)), trace=_trace)
    _CACHE["last_results"] = res
    out = np.empty((B, D2, H, W), np.float32)
    W2 = W - JH * CT        # 64: width of the CT16 quadrant
    for core in range(8):
        b, half = core // 2, core % 2
        o = np.asarray(res.results[core]["out48"])  # [HH, W, SLOT] bf16
        o = o[:, :, :RUN].astype(np.float32).reshape(HH, W, ND, UW)[:, :, :, :ND]
        full = o.transpose(2, 3, 0, 1).reshape(D2, HH, W)
        # rows 16-32 x cols 64-128 live in out56 (8x16-rect layout, UW=56)
        o2 = np.asarray(res.results[core]["out56"])  # [16, 64, SLOT2]
        o2 = o2[:, :, :ND * 56].astype(np.float32).reshape(
            AH, W2, ND, 56)[:, :, :, :ND]
        full[:, AH:, JH * CT:] = o2.transpose(2, 3, 0, 1).reshape(D2, AH, W2)
        out[b, :, HH * half:HH * (half + 1), :] = full
    return out


if __name__ == "__main__":
    x = np.random.randn(B, C, H, W).astype(np.float32)
    y = kernel(x)
    print("kernel output shape:", y.shape)
